# revision 39
# baseline (speedup 1.0000x reference)
"""GQA causal prefill attention on 8 TRN2 NeuronCores.

Baseline schedule skeleton + host-side input prep: q pre-transposed to
qT (4, 128, 2048) bf16, k to kT (128, 2048) bf16, v kept (2048, 128) bf16.
All on-device PE transposes and DVE casts are gone; input DMAs run on three
parallel DGE queues in reverse-chunk order so head 0 (j descending) starts
after the first chunks land.

On top of the baseline schedule: adjacent big-j score tiles ({10,11}, {12,13},
{14,15}) share one 12-tile PSUM unit and a single scalar-engine exp
instruction (the ACT pipe-fill is ~300ns per instruction and ACT is the
busiest engine); dummy identity matmuls keep the PE's HAM clock-gate warm
during the input-DMA window; the output is written as bf16 (half the DMA
bytes) and upcast on the host."""

import sys
import functools

import numpy as np

if "/opt/trn_rl_repo" not in sys.path:
    sys.path.insert(0, "/opt/trn_rl_repo")

T = 2048
H_TOTAL = 32
N_CORES = 8
H = H_TOTAL // N_CORES  # 4 q heads per core
D = 128
P = 128
NT = T // P  # 16 token tiles
SCALE = 0.08838834764831845

_EOFF = [0] * (NT + 1)
for _j in range(NT):
    _EOFF[_j + 1] = _EOFF[_j] + (T - P * _j)
E_COLS = _EOFF[NT]  # 17408


def _n_chunks(n_tiles):
    out = []
    i = 0
    while i < n_tiles:
        c = min(4, n_tiles - i)
        out.append((i, c))
        i += c
    return out


def _build_body(tc, nc, q_d, k_d, v_d, o_d, ctx):
    from collections import deque

    import concourse.mybir as mybir
    from concourse.masks import make_identity, make_upper_triangular

    f32 = mybir.dt.float32
    bf16 = mybir.dt.bfloat16

    const = ctx.enter_context(tc.tile_pool(name="const", bufs=1))
    qbp = ctx.enter_context(tc.tile_pool(name="qbf", bufs=4))
    qtp = ctx.enter_context(tc.tile_pool(name="qT", bufs=4))
    ep = ctx.enter_context(tc.tile_pool(name="eT", bufs=2))
    outp = ctx.enter_context(tc.tile_pool(name="outt", bufs=4))
    recp = ctx.enter_context(tc.tile_pool(name="rec", bufs=4))

    st_pool = ctx.enter_context(tc.tile_pool(name="st", bufs=2, space="PSUM"))
    sm_pool = ctx.enter_context(tc.tile_pool(name="smp", bufs=2, space="PSUM"))

    identity = const.tile([P, P], bf16, tag="ident")
    make_identity(nc, identity)
    utri = const.tile([P, P], bf16, tag="utri")
    make_upper_triangular(nc, utri, val=1.0, diag=True)

    o_view = o_d.rearrange("(i p) h d -> p i h d", p=P)

    warm_sb = recp.tile([P, 1], f32, tag="rec", name="warm")

    kT = const.tile([P, NT, P], bf16, tag="kT")        # [d, j, s]
    v_aug = const.tile([P, NT, D + 1], bf16, tag="vaug")
    qT = [
        qtp.tile([P, NT, P], bf16, tag="qT", name=f"qT{h}") for h in range(H)
    ]
    nc.vector.memset(v_aug[:, :, D:D + 1], 1.0)

    k_view = k_d.rearrange("d (j p) -> d j p", p=P)
    q_view = q_d.rearrange("h d (i p) -> h d i p", p=P)
    v_view = v_d.rearrange("(j p) d -> p j d", p=P)
    CHUNK_ORDER = [3, 2, 0, 1]
    for b in CHUNK_ORDER:
        nc.sync.dma_start(kT[:, 4 * b:4 * b + 4, :], k_view[:, 4 * b:4 * b + 4, :])
        nc.scalar.dma_start(qT[0][:, 4 * b:4 * b + 4, :], q_view[0, :, 4 * b:4 * b + 4, :])
    nc.sync.dma_start(v_aug[:, :, 0:D], v_view)
    for h in range(1, H):
        nc.gpsimd.dma_start(qT[h], q_view[h])

    # ACT table prewarm after the scalar queue's DMA issues
    nc.scalar.activation(
        out=warm_sb, in_=identity[:, 0:1],
        func=mybir.ActivationFunctionType.Exp,
    )

    # PE warmup: real matmuls on the identity keep the HAM clock gate fed
    # while the first input chunks land (transposes don't count as PE-busy).
    warm_mm = sm_pool.tile([P, P], f32, tag="sm", name="warmmm")
    for _ in range(28):
        nc.tensor.matmul(warm_mm, lhsT=identity, rhs=identity,
                         start=True, stop=True)

    def emit_chain(eT, h, i):
        pv = sm_pool.tile([P, P + 1], f32, tag="sm")
        for j in range(i + 1):
            c0 = _EOFF[j] + (i - j) * P
            nc.tensor.matmul(
                pv,
                lhsT=eT[:, c0:c0 + P],
                rhs=v_aug[:, j, :],
                start=(j == 0),
                stop=(j == i),
            )
        rec = recp.tile([P, 1], f32, tag="rec")
        nc.vector.reciprocal(rec, pv[:, D:D + 1])
        ot = outp.tile([P, D], bf16, tag="outt")
        nc.vector.tensor_scalar_mul(ot, pv[:, 0:D], rec)
        nc.sync.dma_start(o_view[:, i, h, :], ot)

    ready = deque()

    def pop_ready(budget, force=False):
        while ready:
            e2, h2, i2 = ready[0]
            size = i2 + 1
            if not force and size > budget and budget < 16:
                break
            ready.popleft()
            emit_chain(e2, h2, i2)
            budget -= size
            if budget <= 0 and not force:
                break

    ST_TILES = 12

    # j-groups: adjacent big-j pairs share one 12-tile PSUM unit and ONE exp
    # instruction (halves the ACT pipe-fill overhead out there); small j keep
    # the baseline's split-into-two-units shape.
    def _groups(h):
        if h == 0:
            return [[14, 15], [12, 13], [10, 11], [9], [8]] + [
                [j] for j in range(0, 8)
            ]
        return [[j] for j in range(0, 10)] + [[10, 11], [12, 13], [14, 15]]

    for h in range(H):
        eT = ep.tile([P, E_COLS], bf16, tag="eT")
        for grp in _groups(h):
            pop_ready((NT - grp[0]) + (8 if h == H - 1 else 4))
            ntiles = sum(NT - j for j in grp)
            if ntiles > ST_TILES:
                # single big j: split across two units as in the baseline
                j = grp[0]
                g0 = (ntiles + 1) // 2
                for (gb, gn) in ((0, g0), (g0, ntiles - g0)):
                    stu = st_pool.tile([P, ST_TILES * P], f32, tag="st")
                    for (i0, ci) in _n_chunks(gn):
                        nc.tensor.matmul(
                            stu[:, i0 * P:(i0 + ci) * P],
                            lhsT=kT[:, j, :],
                            rhs=qT[h][:, j + gb + i0:j + gb + i0 + ci, :],
                            start=True,
                            stop=True,
                        )
                    nc.scalar.activation(
                        out=eT[:, _EOFF[j] + gb * P:_EOFF[j] + (gb + gn) * P],
                        in_=stu[:, 0:gn * P],
                        func=mybir.ActivationFunctionType.Exp,
                        scale=SCALE,
                    )
            else:
                stu = st_pool.tile([P, ST_TILES * P], f32, tag="st")
                p0 = 0
                for j in grp:
                    n = NT - j
                    i0 = j
                    while n > 0:
                        m = min(4 - (p0 % 4), n)
                        nc.tensor.matmul(
                            stu[:, p0 * P:(p0 + m) * P],
                            lhsT=kT[:, j, :],
                            rhs=qT[h][:, i0:i0 + m, :],
                            start=True,
                            stop=True,
                        )
                        p0 += m
                        i0 += m
                        n -= m
                nc.scalar.activation(
                    out=eT[:, _EOFF[grp[0]]:_EOFF[grp[0]] + ntiles * P],
                    in_=stu[:, 0:ntiles * P],
                    func=mybir.ActivationFunctionType.Exp,
                    scale=SCALE,
                )
            for j in grp:
                off = _EOFF[j]
                nc.vector.tensor_tensor(
                    eT[:, off:off + P],
                    eT[:, off:off + P],
                    utri,
                    mybir.AluOpType.mult,
                )
                if h > 0:
                    ready.append((eT, h, j))
                elif j < 8:
                    ready.append((eT, 0, j))
        if h == 0:
            for i in range(8, NT):
                ready.append((eT, 0, i))
        if h >= 1:
            while ready and ready[0][1] < h:
                e2, h2, i2 = ready.popleft()
                emit_chain(e2, h2, i2)
    pop_ready(0, force=True)


@functools.lru_cache(maxsize=1)
def _build():
    import concourse.tile as tile
    import concourse.mybir as mybir
    from concourse import bacc
    from contextlib import ExitStack

    f32 = mybir.dt.float32
    nc = bacc.Bacc(
        "TRN2",
        target_bir_lowering=False,
        debug=False,
        num_devices=N_CORES,
    )
    bf16 = mybir.dt.bfloat16
    q_d = nc.dram_tensor("q", (H, D, T), bf16, kind="ExternalInput").ap()
    k_d = nc.dram_tensor("k", (D, T), bf16, kind="ExternalInput").ap()
    v_d = nc.dram_tensor("v", (T, D), bf16, kind="ExternalInput").ap()
    o_d = nc.dram_tensor("out", (T, H, D), bf16, kind="ExternalOutput").ap()

    with tile.TileContext(nc) as tc:
        with ExitStack() as ctx:
            _build_body(tc, nc, q_d, k_d, v_d, o_d, ctx)
    nc.compile()
    return nc


def _in_maps(q, k, v):
    import ml_dtypes

    bf16 = ml_dtypes.bfloat16
    q = np.asarray(q, dtype=np.float32)
    k = np.asarray(k, dtype=np.float32)
    v = np.asarray(v, dtype=np.float32)
    maps = []
    for c in range(N_CORES):
        qt = np.ascontiguousarray(
            q[:, H * c:H * c + H, :].transpose(1, 2, 0)
        ).astype(bf16)
        kt = np.ascontiguousarray(k[:, c, :].T).astype(bf16)
        vc = np.ascontiguousarray(v[:, c, :]).astype(bf16)
        maps.append({"q": qt, "k": kt, "v": vc})
    return maps


def kernel(q, k, v, _trace=False):
    from concourse.bass_utils import run_bass_kernel_spmd

    nc = _build()
    res = run_bass_kernel_spmd(
        nc, _in_maps(q, k, v), core_ids=list(range(N_CORES)), trace=_trace
    )
    out = np.empty((T, H_TOTAL, D), dtype=np.float32)
    for c in range(N_CORES):
        out[:, H * c:H * c + H, :] = np.asarray(
            res.results[c]["out"], dtype=np.float32
        ).reshape(T, H, D)
    if _trace:
        return out, res
    return out
